# revision 35
# baseline (speedup 1.0000x reference)
"""Multi-head latent attention (MLA-style) Trainium2 kernel, 8-core SPMD.

Sharding: tensor-parallel over heads. Core c computes heads (2c, 2c+1):
  - kv latent (Wdkv) replicated per core (computed from x on-chip)
  - per-head compressed q, latent-space causal attention, and the head's
    slice of the output projection (row-sharded out_w)
  - per-core output is a PARTIAL [B*T, C] sum; host adds the 8 partials
    and the output bias.

All matmuls run in bf16 (fp32 PSUM accumulation).

Layouts (host-prepared):
  xT     [B, 8, 128, T]    x[b].T              (c = o*128 + p)
  lw     [8, 128, 289]     latent_w, zero-padded col 288
  lbt    [128, 3]          latent_b per l-tile (fp32)
  wd     [8, 128, 576]     Wd_w[h]/8 for the core's 2 heads, h*288+l
  wdbt   [128, 6]          Wd_b[h]/8 per (h, l-tile) (fp32)
  ow     [6, 128, 1024]    out_w rows per (h, l-tile), zero-padded
  masks  [4, 128, 512]     causal masks for the 4 diagonal key tiles
  id128  [128, 128]        identity (PE transpose)
Output:
  out_p  [4096, 1024] fp32 partial
"""

import numpy as np
import ml_dtypes

B, T, C = 2, 2048, 1024
H, L = 16, 288
NCORES = 8
HPC = H // NCORES  # heads per core
BT = B * T

# l-dimension tiles of L=288 (and the +1 sum row for the y matmul)
LT = [(0, 128), (1, 128), (2, 32)]
MT = [(0, 128), (1, 128), (2, 33)]  # y-matmul M tiles (includes sum row 288)

_cache = {}


def _build_nc():
    import concourse.bacc as bacc
    import concourse.mybir as mybir
    import concourse.tile as tile
    from concourse.bass import ts

    bf16 = mybir.dt.bfloat16
    f32 = mybir.dt.float32

    nc = bacc.Bacc("TRN2", target_bir_lowering=False, debug=True)

    d_xT = nc.dram_tensor("xT", [B, 8, 128, T], bf16, kind="ExternalInput")
    d_lw = nc.dram_tensor("lw", [8, 128, 289], bf16, kind="ExternalInput")
    d_lbt = nc.dram_tensor("lbt", [128, 3], f32, kind="ExternalInput")
    d_wd = nc.dram_tensor("wd", [8, 128, 576], bf16, kind="ExternalInput")
    d_wd2 = nc.dram_tensor("wd2", [8, 128, 64], bf16, kind="ExternalInput")
    d_wdbt = nc.dram_tensor("wdbt", [128, 6], f32, kind="ExternalInput")
    d_wdbt2 = nc.dram_tensor("wdbt2", [64, 1], f32, kind="ExternalInput")
    d_ow = nc.dram_tensor("ow", [6, 128, 1024], bf16, kind="ExternalInput")
    d_ow2 = nc.dram_tensor("ow2", [64, 1024], bf16, kind="ExternalInput")
    d_masks = nc.dram_tensor("masks", [4, 128, 512], bf16, kind="ExternalInput")
    d_id = nc.dram_tensor("id128", [128, 128], bf16, kind="ExternalInput")
    d_out = nc.dram_tensor("out_p", [BT, C], f32, kind="ExternalOutput")

    Exp = mybir.ActivationFunctionType.Exp
    Ident = mybir.ActivationFunctionType.Identity

    with tile.TileContext(nc) as tc:
        with (
            tc.tile_pool(name="const", bufs=1) as cpool,
            tc.tile_pool(name="xp", bufs=2) as xpool,
            tc.tile_pool(name="kvp", bufs=2) as kvpool,
            tc.tile_pool(name="qp", bufs=2) as qpool,
            tc.tile_pool(name="ep", bufs=4) as epool,
            tc.tile_pool(name="yp", bufs=2) as ypool,
            tc.tile_pool(name="rp", bufs=1) as rpool,
            tc.tile_pool(name="op", bufs=3) as opool,
            tc.tile_pool(name="ps_y", bufs=1, space="PSUM") as ps_y,
            tc.tile_pool(name="ps_s", bufs=3, space="PSUM") as ps_s,
            tc.tile_pool(name="ps_m", bufs=2, space="PSUM") as ps_m,
        ):
            # ---- persistent weights ----
            # latent_w first: the kvT matmuls only need lw + the first x
            # chunk, so the PE can start ~10us earlier
            lw_sb = cpool.tile([128, 8, 289], bf16, name="lw_sb")
            for kc in range(8):
                nc.sync.dma_start(lw_sb[:, kc, :], d_lw[kc])
            lbt_sb = cpool.tile([128, 3], f32, name="lbt_sb")
            nc.sync.dma_start(lbt_sb[:], d_lbt[:])
            id_sb = cpool.tile([128, 128], bf16, name="id_sb")
            nc.sync.dma_start(id_sb[:], d_id[:])
            wd_sb = cpool.tile([128, 8, 576], bf16, name="wd_sb")
            wd2_sb = cpool.tile([128, 8, 64], bf16, name="wd2_sb")
            wdbt_sb = cpool.tile([128, 6], f32, name="wdbt_sb")
            wdbt2_sb = cpool.tile([64, 1], f32, name="wdbt2_sb")
            ow_sb = cpool.tile([128, 6, 1024], bf16, name="ow_sb")
            ow2_sb = cpool.tile([64, 1024], bf16, name="ow2_sb")
            masks_sb = cpool.tile([128, 4, 512], bf16, name="masks_sb")

            def load_weights():
                for kc in range(8):
                    nc.sync.dma_start(wd_sb[:, kc, :], d_wd[kc])
                    nc.sync.dma_start(wd2_sb[:, kc, :], d_wd2[kc])
                nc.sync.dma_start(wdbt_sb[:], d_wdbt[:])
                nc.sync.dma_start(wdbt2_sb[:], d_wdbt2[:])
                for i in range(6):
                    nc.sync.dma_start(ow_sb[:, i, :], d_ow[i])
                nc.sync.dma_start(ow2_sb[:], d_ow2[:])
                for i in range(4):
                    nc.sync.dma_start(masks_sb[:, i, :], d_masks[i])

            # deferred out-projection: (yts, b, qc) emitted one head late so
            # the PE queue never blocks on the normalize chain
            pending = []

            def emit_outproj():
                yts, yt2s, pb, pqc = pending.pop()
                for blk in range(4):
                    osb = opool.tile([128, 1024], f32, name="osb", tag="osb", bufs=2)
                    for cc in range(2):
                        po = ps_m.tile([128, 512], f32, name="ps_o", tag="m")
                        for h in range(HPC):
                            for lt in (0, 1):
                                nc.tensor.matmul(
                                    po,
                                    yts[h][:, lt, ts(blk, 128)],
                                    ow_sb[:, h * 3 + lt, ts(cc, 512)],
                                    start=(h == 0 and lt == 0),
                                    stop=False,
                                )
                        # both heads' l2 blocks stacked into one K=64 matmul
                        nc.tensor.matmul(
                            po,
                            yt2s[:, ts(blk, 128)],
                            ow2_sb[:, ts(cc, 512)],
                            start=False,
                            stop=True,
                        )
                        nc.vector.tensor_copy(osb[:, ts(cc, 512)], po[:])
                    row0 = pb * T + pqc * 512 + blk * 128
                    nc.sync.dma_start(d_out[row0 : row0 + 128, :], osb[:])

            # the last chunk's out-projection is emitted per head (h0's half
            # overlaps h1's attention) to shrink the end-of-kernel tail
            def emit_final_h0(yt0, yt2c0):
                osbs = []
                for blk in range(4):
                    osb = opool.tile(
                        [128, 1024], f32, name="osbf", tag="osbf", bufs=4
                    )
                    for cc in range(2):
                        po = ps_m.tile([128, 512], f32, name="ps_o", tag="m")
                        for lt in (0, 1):
                            nc.tensor.matmul(
                                po,
                                yt0[:, lt, ts(blk, 128)],
                                ow_sb[:, lt, ts(cc, 512)],
                                start=(lt == 0),
                                stop=False,
                            )
                        nc.tensor.matmul(
                            po,
                            yt2c0[:, ts(blk, 128)],
                            ow_sb[:32, 2, ts(cc, 512)],
                            start=False,
                            stop=True,
                        )
                        nc.vector.tensor_copy(osb[:, ts(cc, 512)], po[:])
                    osbs.append(osb)
                return osbs

            def emit_final_h1(yt1, yt2c1, osbs, pb, pqc):
                for blk in range(4):
                    for cc in range(2):
                        po = ps_m.tile([128, 512], f32, name="ps_o", tag="m")
                        for lt in (0, 1):
                            nc.tensor.matmul(
                                po,
                                yt1[:, lt, ts(blk, 128)],
                                ow_sb[:, 3 + lt, ts(cc, 512)],
                                start=(lt == 0),
                                stop=False,
                            )
                        nc.tensor.matmul(
                            po,
                            yt2c1[:, ts(blk, 128)],
                            ow_sb[:32, 5, ts(cc, 512)],
                            start=False,
                            stop=True,
                        )
                        nc.vector.tensor_add(
                            osbs[blk][:, ts(cc, 512)],
                            po[:],
                            osbs[blk][:, ts(cc, 512)],
                        )
                    row0 = pb * T + pqc * 512 + blk * 128
                    nc.sync.dma_start(d_out[row0 : row0 + 128, :], osbs[blk][:])

            for b in range(B):
                # ---- load x[b]^T, per 512-chunk ----
                xts = []
                for tch in range(4):
                    xt = xpool.tile([128, 8, 512], bf16, name="xt", tag=f"xT{tch}")
                    for o in range(8):
                        nc.sync.dma_start(
                            xt[:, o, :], d_xT[b, o][:, ts(tch, 512)]
                        )
                    xts.append(xt)
                if b == 0:
                    load_weights()

                # ---- kvT = (x @ latent_w + latent_b)^T : [l, t], per chunk;
                #      kv_aug[t, 0:289] = [kv | 1] via PE transpose ----
                kvts, kvas, kv2ps = [], [], []
                for tch in range(4):
                    kvt = kvpool.tile([128, 3, 512], bf16, name="kvt", tag=f"kvT{tch}")
                    for lt, lsz in LT:
                        pq = ps_s.tile([128, 512], f32, name="ps_kv", tag="s")
                        for kc in range(8):
                            nc.tensor.matmul(
                                pq[:lsz],
                                lw_sb[:, kc, lt * 128 : lt * 128 + lsz],
                                xts[tch][:, kc, :],
                                start=(kc == 0),
                                stop=(kc == 7),
                            )
                        nc.scalar.activation(
                            kvt[:lsz, lt, :],
                            pq[:lsz],
                            Ident,
                            bias=lbt_sb[:lsz, lt : lt + 1],
                        )
                    kvts.append(kvt)

                    # kv-l2 relaid out so adjacent t-tiles sit at partition
                    # offsets 0/32, enabling paired (concurrent) K=32 matmuls
                    kv2p = kvpool.tile([64, 2, 128], bf16, name="kv2p", tag=f"kv2p{tch}")
                    for j in range(4):
                        nc.sync.dma_start(
                            kv2p[32 * (j % 2) : 32 * (j % 2) + 32, j // 2, :],
                            kvt[:32, 2, ts(j, 128)],
                        )
                    kv2ps.append(kv2p)

                    kva = kvpool.tile([128, 4, 289], bf16, name="kva", tag=f"kva{tch}")
                    for tt in range(4):
                        nc.vector.memset(kva[:, tt, 288:289], 1.0)
                        for lt, lsz in LT:
                            pt = ps_m.tile([128, 512], bf16, name="ps_t", tag="m")
                            nc.tensor.transpose(
                                pt[:, :lsz],
                                kvt[:lsz, lt, ts(tt, 128)],
                                id_sb[:lsz, :lsz],
                            )
                            nc.vector.tensor_copy(
                                kva[:, tt, lt * 128 : lt * 128 + lsz], pt[:, :lsz]
                            )
                    kvas.append(kva)

                # ---- attention per (chunk, head) ----
                for qc in range(4):
                    final = b == B - 1 and qc == 3
                    yts = []
                    yt2s = None
                    if not final:
                        yt2s = ypool.tile([64, 512], bf16, name="yt2s", tag="yt2")

                    # both heads' l2 (l=256..287) q-projection stacked into
                    # one M=64 matmul group; h1's half is DMA-shifted back to
                    # partition 0 so the scores matmul K ranges line up
                    pq2 = ps_s.tile([128, 512], f32, name="ps_q2", tag="s")
                    for kc in range(8):
                        nc.tensor.matmul(
                            pq2[:64],
                            wd2_sb[:, kc, :],
                            xts[qc][:, kc, :],
                            start=(kc == 0),
                            stop=(kc == 7),
                        )
                    qt2w = qpool.tile([64, 512], bf16, name="qt2w", tag="qt2w")
                    nc.scalar.activation(
                        qt2w[:], pq2[:64], Ident, bias=wdbt2_sb[:, 0:1]
                    )
                    qt2b = qpool.tile([32, 512], bf16, name="qt2b", tag="qt2b")
                    nc.sync.dma_start(qt2b[:], qt2w[32:64, :])
                    # h0's l2 q replicated at partition offset 32 (pair partner)
                    qt0alt = qpool.tile([64, 512], bf16, name="qt0alt", tag="qt0alt")
                    nc.sync.dma_start(qt0alt[32:64, :], qt2w[:32, :])

                    for h in range(HPC):
                        # q^T chunk [l, 512] (scale 1/8 folded into wd)
                        qt = qpool.tile([128, 2, 512], bf16, name="qt", tag="qt")
                        for lt in (0, 1):
                            pq = ps_s.tile([128, 512], f32, name="ps_q", tag="s")
                            for kc in range(8):
                                nc.tensor.matmul(
                                    pq,
                                    wd_sb[:, kc, h * 288 + lt * 128 :][:, :128],
                                    xts[qc][:, kc, :],
                                    start=(kc == 0),
                                    stop=(kc == 7),
                                )
                            nc.scalar.activation(
                                qt[:, lt, :],
                                pq[:],
                                Ident,
                                bias=wdbt_sb[:, h * 3 + lt : h * 3 + lt + 1],
                            )


                        # scores^T -> exp -> (mask) -> y accumulation
                        py = [
                            ps_y.tile([128, 512], f32, name=f"ps_y{mt}", tag=f"y{mt}")
                            for mt, _ in MT
                        ]
                        ntk = qc * 4 + 4

                        def emit_y(tk, et, c0):
                            for mt, msz in MT:
                                nc.tensor.matmul(
                                    py[mt][:msz, c0:],
                                    kvas[tk // 4][:, tk % 4, mt * 128 :][:, :msz],
                                    et[:, c0:],
                                    start=(tk == 0),
                                    stop=(tk == ntk - 1),
                                )

                        # scores/exp pipelined one pair ahead of the y matmuls
                        # so the PE queue never blocks on the ACT exp; the two
                        # K=32 l2 matmuls of each pair run in concurrent PE
                        # row groups (partition offsets 0 / 32)
                        pend = []
                        for pr in range(ntk // 2):
                            pair = []
                            for tk in (2 * pr, 2 * pr + 1):
                                # diagonal tiles: only columns >= c0 unmasked
                                c0 = max(0, (tk - qc * 4) * 128)
                                pss = ps_s.tile(
                                    [128, 512], f32, name="ps_s", tag="s"
                                )
                                for lt in (0, 1):
                                    nc.tensor.matmul(
                                        pss[:, c0:],
                                        kvts[tk // 4][:, lt, ts(tk % 4, 128)],
                                        qt[:, lt, c0:],
                                        start=(lt == 0),
                                        stop=False,
                                    )
                                pair.append((tk, pss, c0))
                            for off, (tk, pss, c0) in zip((0, 32), pair):
                                if off == 0:
                                    rhs = (qt2w if h == 0 else qt2b)[:32, c0:]
                                else:
                                    rhs = (qt0alt if h == 0 else qt2w)[
                                        32:64, c0:
                                    ]
                                nc.tensor.matmul(
                                    pss[:, c0:],
                                    kv2ps[tk // 4][
                                        off : off + 32, (tk % 4) // 2, :
                                    ],
                                    rhs,
                                    start=False,
                                    stop=True,
                                )
                            for tk, pss, c0 in pair:
                                et = epool.tile(
                                    [128, 512], bf16, name="et", tag="et"
                                )
                                nc.scalar.activation(et[:, c0:], pss[:, c0:], Exp)
                                i = tk - qc * 4
                                if i >= 0:
                                    # mask is nontrivial only in the i-th
                                    # 128-column block
                                    nc.vector.tensor_mul(
                                        et[:, c0 : c0 + 128],
                                        et[:, c0 : c0 + 128],
                                        masks_sb[:, i, c0 : c0 + 128],
                                    )
                                pend.append((tk, et, c0))
                            while len(pend) > 2:
                                emit_y(*pend.pop(0))
                            if final and h == 1 and pr == 3:
                                # h0's deferred out-projection, emitted here so
                                # its matmuls enter the PE queue well after
                                # h0's normalize chain has completed
                                final_osbs = emit_final_h0(*final_h0_args)
                        for e in pend:
                            emit_y(*e)

                        # copy y accumulators to SBUF right away so the PSUM
                        # banks free for the next head/chunk without waiting
                        # on the normalize chain
                        ysb = ypool.tile([128, 3, 512], f32, name="ysb", tag="ysb", bufs=1)
                        for mt, msz in MT:
                            nc.scalar.copy(ysb[:msz, mt, :], py[mt][:msz])
                        # normalize: r = 1/sum, broadcast, scale yT
                        r_sb = rpool.tile([1, 512], f32, name="r_sb", tag="r")
                        nc.vector.reciprocal(r_sb[:], ysb[32:33, 2, :])
                        rb_sb = rpool.tile([128, 512], f32, name="rb_sb", tag="rb")
                        nc.gpsimd.partition_broadcast(rb_sb[:], r_sb[:1, :])
                        yt = ypool.tile([128, 2, 512], bf16, name="yt", tag=f"yt{h}")
                        for lt in (0, 1):
                            nc.vector.tensor_mul(
                                yt[:, lt, :], ysb[:, lt, :], rb_sb[:]
                            )
                        if final:
                            yt2c = ypool.tile(
                                [32, 512], bf16, name="yt2c", tag=f"yt2c{h}"
                            )
                            nc.vector.tensor_mul(yt2c[:], ysb[:32, 2, :], rb_sb[:32])
                        else:
                            nc.vector.tensor_mul(
                                yt2s[h * 32 : (h + 1) * 32, :],
                                ysb[:32, 2, :],
                                rb_sb[:32],
                            )
                        yts.append(yt)

                        # out-projection deferred by one head
                        if pending:
                            emit_outproj()
                        if final:
                            if h == 0:
                                final_h0_args = (yt, yt2c)
                            else:
                                emit_final_h1(yt, yt2c, final_osbs, b, qc)
                    if not final:
                        pending.append((yts, yt2s, b, qc))

    nc.finalize()
    return nc


def _get_nc():
    if "nc" not in _cache:
        _cache["nc"] = _build_nc()
    return _cache["nc"]


def _prep_inputs(x, latent_w, latent_b, Wd_w, Wd_b, out_w):
    """Host-side shard + layout prep. Returns list of 8 per-core input maps."""
    bf16 = ml_dtypes.bfloat16
    x = np.asarray(x, dtype=np.float32)
    latent_w = np.asarray(latent_w, dtype=np.float32)
    latent_b = np.asarray(latent_b, dtype=np.float32)
    Wd_w = np.asarray(Wd_w, dtype=np.float32)
    Wd_b = np.asarray(Wd_b, dtype=np.float32)
    out_w = np.asarray(out_w, dtype=np.float32)

    xT = np.ascontiguousarray(x.transpose(0, 2, 1)).reshape(B, 8, 128, T)
    xT = xT.astype(bf16)

    lw = np.zeros((C, 289), np.float32)
    lw[:, :288] = latent_w
    lw = lw.reshape(8, 128, 289).astype(bf16)

    lbt = np.zeros((128, 3), np.float32)
    for lt, lsz in LT:
        lbt[:lsz, lt] = latent_b[lt * 128 : lt * 128 + lsz]

    id128 = np.eye(128, dtype=np.float32).astype(bf16)

    # causal masks for the 4 diagonal key tiles: mask[i][tk, tq] = tq >= i*128+tk
    tq = np.arange(512)[None, :]
    tk = np.arange(128)[:, None]
    masks = np.stack([(tq >= i * 128 + tk) for i in range(4)]).astype(np.float32)
    masks = masks.astype(bf16)

    in_maps = []
    for c in range(NCORES):
        heads = [HPC * c + i for i in range(HPC)]
        wd = np.zeros((8, 128, 576), np.float32)
        wd2 = np.zeros((8, 128, 64), np.float32)
        wdbt = np.zeros((128, 6), np.float32)
        wdbt2 = np.zeros((64, 1), np.float32)
        ow = np.zeros((6, 128, 1024), np.float32)
        ow2 = np.zeros((64, 1024), np.float32)
        for i, h in enumerate(heads):
            ow2[i * 32 : (i + 1) * 32, :] = out_w[h * 288 + 256 : h * 288 + 288, :]
            wd2[:, :, i * 32 : (i + 1) * 32] = (
                Wd_w[h][:, 256:288] / 8.0
            ).reshape(8, 128, 32)
            wdbt2[i * 32 : (i + 1) * 32, 0] = Wd_b[h][256:288] / 8.0
            wd[:, :, i * 288 : (i + 1) * 288] = (Wd_w[h] / 8.0).reshape(8, 128, 288)
            for lt, lsz in LT:
                wdbt[:lsz, i * 3 + lt] = Wd_b[h][lt * 128 : lt * 128 + lsz] / 8.0
                ow[i * 3 + lt, :lsz, :] = out_w[
                    h * 288 + lt * 128 : h * 288 + lt * 128 + lsz, :
                ]
        in_maps.append(
            {
                "xT": xT,
                "lw": lw,
                "lbt": lbt,
                "wd": wd.astype(bf16),
                "wd2": wd2.astype(bf16),
                "wdbt": wdbt,
                "wdbt2": wdbt2,
                "ow": ow.astype(bf16),
                "ow2": ow2.astype(bf16),
                "masks": masks,
                "id128": id128,
            }
        )
    return in_maps


def kernel(x, latent_w, latent_b, Wd_w, Wd_b, out_w, out_b, **kw):
    from concourse import bass_utils

    nc = _get_nc()
    in_maps = _prep_inputs(x, latent_w, latent_b, Wd_w, Wd_b, out_w)
    res = bass_utils.run_bass_kernel_spmd(nc, in_maps, core_ids=list(range(NCORES)))
    out = np.zeros((BT, C), np.float64)
    for c in range(NCORES):
        out += res.results[c]["out_p"].astype(np.float64)
    out += np.asarray(out_b, dtype=np.float64)[None, :]
    return out.reshape(B, T, C).astype(np.float32)
